# revision 15
# baseline (speedup 1.0000x reference)
"""Trainium2 Bass kernel for a CSDE decoder (Euler-Maruyama scan with
step-aware GLU drift + diffusion MLP), data-parallel over batch on 8 cores.

Math per step k (reference):
    x   = concat(z, y[:, :, None], -1)                # [B,N,65]
    a   = x @ Wa_k + ba_k ;  b = x @ Wb_k + bb_k      # [B,N,64]
    f   = (a * sigmoid(b)) @ wo_k + bo_k              # [B,N]
    g   = tanh(relu(relu(relu(y@GW0+Gb0)@GW1+Gb1)@GW2+Gb2)@GWf+Gbf)
    y  <- y + f*dt + g*sqdt*eps_k

Device layout (per core, B_c=128 batch rows):
  - tokens = (node, batch) pairs, node-major columns: col = n*128 + b
  - xT resident in SBUF as [65, 328*128] bf16: rows 0-63 = z^T (static),
    row 64 = y broadcast row, re-written each step via a spread DMA.
  - wo*dt folded into Wa / ba / bo on the host.
  - per 8-node group: 4 matmuls (a/b for 2 half-groups stacked on
    partitions) -> sigmoid (ACT, full width) -> glu (DVE) -> block-diag
    ones matmul reduces glu over H to f for all 8 nodes.
  - g_func runs on y^T chunks ([node,batch] layout), all matmuls bf16.
  - y state kept fp32 in [128, 3*128] chunk layout; updates on DVE.
Host pre-transposes z / noise / y0 into these layouts (pure np prep).
"""

import sys

import numpy as np

sys.path.insert(0, "/opt/trn_rl_repo")

import ml_dtypes

import concourse.bass as bass
import concourse.bacc as bacc
import concourse.tile as tile
from concourse import mybir
from concourse import bass_utils
from contextlib import ExitStack

BF16 = mybir.dt.bfloat16
F32 = mybir.dt.float32

LAST_RESULTS = None  # BassKernelResults of the most recent kernel() call

B, N, H, HH, T = 1024, 325, 64, 64, 12
NCORES = 8
BC = B // NCORES          # 128 batch rows per core
NP = 328                  # nodes padded to 8*41
NG = NP // 8              # 41 groups of 8 nodes, exactly 16 per 128-node chunk
NPC = NP * 128            # valid xT columns
CHP = [128, 128, 72]      # packed chunk row counts (incl pad nodes)
C2R = N - 256             # 69 real rows in chunk 2


def _build_program(bo_dt: np.ndarray, sqdt: float):
    """Emit the full 12-step SPMD program for one core."""
    nc = bacc.Bacc("TRN2", target_bir_lowering=False, debug=False,
                   num_devices=NCORES)

    xtinit_d = nc.declare_dram_parameter("xtinit", [65, 3 * 16384], BF16, isOutput=False)
    y0t_d = nc.declare_dram_parameter("y0t", [128, 384], F32, isOutput=False)
    noiset_d = nc.declare_dram_parameter("noiset", [T, 128, 384], F32, isOutput=False)
    wab_d = nc.declare_dram_parameter("wab", [65, 128 * T], BF16, isOutput=False)
    bstack_d = nc.declare_dram_parameter("bstack", [128, 2 * T], F32, isOutput=False)
    gw0_d = nc.declare_dram_parameter("gw0", [128, 192], BF16, isOutput=False)
    gw12_d = nc.declare_dram_parameter("gw12", [64, 128], BF16, isOutput=False)
    gwf_d = nc.declare_dram_parameter("gwf", [64, NP], BF16, isOutput=False)
    gb_d = nc.declare_dram_parameter("gb", [128, 8], F32, isOutput=False)
    ones2_d = nc.declare_dram_parameter("ones2", [128, 32], BF16, isOutput=False)
    yout_d = nc.declare_dram_parameter("yout", [128, 384], F32, isOutput=True)

    Sig = mybir.ActivationFunctionType.Sigmoid
    Relu = mybir.ActivationFunctionType.Relu
    Tanh = mybir.ActivationFunctionType.Tanh
    ADD = mybir.AluOpType.add
    MULT = mybir.AluOpType.mult

    with ExitStack() as ctx:
        tc = ctx.enter_context(tile.TileContext(nc))
        pers = ctx.enter_context(tc.tile_pool(name="pers", bufs=1))

        xTs = [pers.tile([65, 16384], BF16, name=f"xT{c}") for c in range(3)]
        wab = pers.tile([65, 128 * T], BF16)
        bstack = pers.tile([128, 2 * T], F32)
        gw0 = pers.tile([128, 192], BF16)
        gw12 = pers.tile([64, 128], BF16)
        gwf = pers.tile([64, NP], BF16)
        gb = pers.tile([128, 8], F32)
        ones2 = pers.tile([128, 32], BF16)

        for c in range(3):
            nc.sync.dma_start(xTs[c][:, :], xtinit_d[:, 16384 * c:16384 * c + 16384])
        nc.sync.dma_start(wab[:, :], wab_d[:, :])
        nc.sync.dma_start(bstack[:, :], bstack_d[:, :])
        nc.sync.dma_start(gw0[:, :], gw0_d[:, :])
        nc.sync.dma_start(gw12[:, :], gw12_d[:, :])
        nc.sync.dma_start(gwf[:, :], gwf_d[:, :])
        nc.sync.dma_start(gb[:, :], gb_d[:, :])
        nc.sync.dma_start(ones2[:, :], ones2_d[:, :])

        ypool = ctx.enter_context(tc.tile_pool(name="ypool", bufs=2))
        ybfpool = ctx.enter_context(tc.tile_pool(name="ybfpool", bufs=2))
        epspool = ctx.enter_context(tc.tile_pool(name="epspool", bufs=3))
        fntpool = ctx.enter_context(tc.tile_pool(name="fntpool", bufs=2))
        gtpool = ctx.enter_context(tc.tile_pool(name="gtpool", bufs=6))
        hpool = ctx.enter_context(tc.tile_pool(name="hpool", bufs=3))
        sigpool = ctx.enter_context(tc.tile_pool(name="sigpool", bufs=3))
        glupool = ctx.enter_context(tc.tile_pool(name="glupool", bufs=3))
        fstpool = ctx.enter_context(tc.tile_pool(name="fstpool", bufs=3))
        upool = ctx.enter_context(tc.tile_pool(name="upool", bufs=4))

        papool = ctx.enter_context(tc.tile_pool(name="papool", bufs=2, space="PSUM"))  # 8 PSUM banks: pa2+pb2+pfa2+pfb1+pg1
        pbpool = ctx.enter_context(tc.tile_pool(name="pbpool", bufs=2, space="PSUM"))
        pfpool = ctx.enter_context(tc.tile_pool(name="pfpool", bufs=2, space="PSUM"))
        pgpool = ctx.enter_context(tc.tile_pool(name="pgpool", bufs=2, space="PSUM"))

        ycur = [ypool.tile([128, 128], F32, name=f"y_init{c}", tag=f"y{c}")
                for c in range(3)]
        ybf = [ybfpool.tile([128, 128], BF16, name=f"ybf_init{c}", tag=f"ybf{c}")
               for c in range(3)]
        for c in range(3):
            nc.sync.dma_start(ycur[c][:, :], y0t_d[:, 128 * c:128 * c + 128])
            nc.vector.tensor_copy(ybf[c][:, :], ycur[c][:, :])
        tc.strict_bb_all_engine_barrier()

        for k in range(T):
            eps = epspool.tile([128, 384], F32, name=f"eps{k}", tag="eps")
            nc.sync.dma_start(eps[:, :], noiset_d[k])
            fnts = [fntpool.tile([128, 128], F32, name=f"fnt{k}_{c}", tag=f"fnt{c}")
                    for c in range(3)]
            nc.vector.memset(fnts[2][64:128, :], 0.0)

            # ---- diffusion MLP g(y) on y^T chunks (tiny, runs early) ----
            h0 = pgpool.tile([128, 128], F32, name=f"h0_{k}", tag="pg", bufs=1)
            nc.tensor.matmul(h0[0:64, :], gw0[0:128, 0:64], ybf[0][:, :],
                             start=True, stop=False)
            nc.tensor.matmul(h0[0:64, :], gw0[0:128, 64:128], ybf[1][:, :],
                             start=False, stop=False)
            nc.tensor.matmul(h0[0:64, :], gw0[0:C2R, 128:192], ybf[2][0:C2R, :],
                             start=False, stop=True)
            h0s = hpool.tile([64, 128], BF16, name=f"h0s{k}", tag="hs")
            nc.scalar.activation(h0s[:, :], h0[0:64, :], Relu, bias=gb[0:64, 0:1])

            h1 = pgpool.tile([128, 128], F32, name=f"h1_{k}", tag="pg", bufs=1)
            nc.tensor.matmul(h1[0:64, :], gw12[:, 0:64], h0s[:, :])
            h1s = hpool.tile([64, 128], BF16, name=f"h1s{k}", tag="hs")
            nc.scalar.activation(h1s[:, :], h1[0:64, :], Relu, bias=gb[0:64, 1:2])

            h2 = pgpool.tile([128, 128], F32, name=f"h2_{k}", tag="pg", bufs=1)
            nc.tensor.matmul(h2[0:64, :], gw12[:, 64:128], h1s[:, :])
            h2s = hpool.tile([64, 128], BF16, name=f"h2s{k}", tag="hs")
            nc.scalar.activation(h2s[:, :], h2[0:64, :], Relu, bias=gb[0:64, 2:3])

            gts = []
            for c in range(3):
                csz = 128 if c < 2 else C2R
                gp = pgpool.tile([128, 128], F32, name=f"gp{k}_{c}", tag="pg", bufs=1)
                nc.tensor.matmul(gp[0:csz, :], gwf[:, 128 * c:128 * c + csz],
                                 h2s[:, :])
                gt = gtpool.tile([128, 128], F32, name=f"gt{k}_{c}", tag="gt")
                if csz < 128:
                    nc.vector.memset(gt[64:128, :], 0.0)
                nc.scalar.activation(gt[0:csz, :], gp[0:csz, :], Tanh,
                                     bias=gb[0:csz, 3 + c:4 + c])
                gts.append(gt)

            # ---- GLU drift: 41 groups of 8 nodes (16 per chunk) ----
            # Half A = even node offsets {0,2,4,6}, half B = odd. f-matmuls
            # j=0..2 land in pFa at bases {0,32,64}; j=3 in pFb at base 0.
            wa_k = wab[:, 128 * k:128 * k + 64]
            wb_k = wab[:, 128 * k + 64:128 * k + 128]
            for g in range(NG):
                c = g // 16
                base = 1024 * (g % 16)
                blk = xTs[c][:, base:base + 1024].rearrange(
                    "p (n two b) -> p two n b", two=2, b=128)
                rhsA = blk[:, 0]          # [65, 4, 128] even node offsets
                rhsB = blk[:, 1]          # odd node offsets
                pa = papool.tile([128, 512], F32, name=f"pa{k}_{g}", tag="pa")
                pb = pbpool.tile([128, 512], F32, name=f"pb{k}_{g}", tag="pb")
                nc.tensor.matmul(pa[0:64, :], wa_k, rhsA)
                nc.tensor.matmul(pa[64:128, :], wa_k, rhsB)
                nc.tensor.matmul(pb[0:64, :], wb_k, rhsA)
                nc.tensor.matmul(pb[64:128, :], wb_k, rhsB)

                sig = sigpool.tile([128, 512], F32, name=f"sig{k}_{g}", tag="sig")
                nc.scalar.activation(sig[:, :], pb[:, :], Sig,
                                     bias=bstack[:, 2 * k + 1:2 * k + 2])
                glu = glupool.tile([128, 512], BF16, name=f"glu{k}_{g}", tag="glu")
                nc.vector.scalar_tensor_tensor(
                    glu[:, :], pa[:, :], bstack[:, 2 * k:2 * k + 1], sig[:, :],
                    op0=ADD, op1=MULT)

                pFa = pfpool.tile([128, 128], F32, name=f"pfa{k}_{g}", tag="pfa")
                pFb = pfpool.tile([128, 128], F32, name=f"pfb{k}_{g}", tag="pfb", bufs=1)
                for j in range(3):
                    nc.tensor.matmul(pFa[32 * j:32 * j + 32, :], ones2[:, :],
                                     glu[:, 128 * j:128 * j + 128])
                nc.tensor.matmul(pFb[0:32, :], ones2[:, :], glu[:, 384:512])
                fst = fstpool.tile([128, 128], F32, name=f"fst{k}_{g}", tag="fst")
                nc.scalar.copy(fst[0:66, :], pFa[0:66, :])
                fst2 = fstpool.tile([2, 128], F32, name=f"fst2_{k}_{g}", tag="fst2")
                nc.scalar.copy(fst2[0:2, :], pFb[0:2, :])
                r0 = 8 * (g % 16)
                for j in range(3):
                    nc.sync.dma_start(fnts[c][r0 + 2 * j:r0 + 2 * j + 2, :],
                                      fst[32 * j:32 * j + 2, :])
                nc.sync.dma_start(fnts[c][r0 + 6:r0 + 8, :], fst2[0:2, :])

            # ---- Euler-Maruyama update per chunk ----
            ynext = [ypool.tile([128, 128], F32, name=f"y{k + 1}_{c}", tag=f"y{c}")
                     for c in range(3)]
            ybfn = [ybfpool.tile([128, 128], BF16, name=f"ybf{k + 1}_{c}",
                                 tag=f"ybf{c}") for c in range(3)]
            for c in range(3):
                cs = slice(128 * c, 128 * c + 128)
                t1 = upool.tile([128, 128], F32, name=f"t1_{k}_{c}", tag="t1")
                nc.vector.tensor_mul(t1[:, :], gts[c][:, :], eps[:, cs])
                t2 = upool.tile([128, 128], F32, name=f"t2_{k}_{c}", tag="t2")
                nc.vector.scalar_tensor_tensor(
                    t2[:, :], t1[:, :], float(sqdt), ycur[c][:, :],
                    op0=MULT, op1=ADD)
                nc.vector.scalar_tensor_tensor(
                    ynext[c][:, :], fnts[c][:, :], float(bo_dt[k]), t2[:, :],
                    op0=ADD, op1=ADD)
                nc.vector.tensor_copy(ybfn[c][:, :], ynext[c][:, :])
                if k < T - 1:
                    nc.sync.dma_start(xTs[c][64:65, :], ybfn[c][:, :])
            ycur = ynext
            ybf = ybfn

        for c in range(3):
            nc.sync.dma_start(yout_d[:, 128 * c:128 * c + 128], ycur[c][:, :])

    nc.compile()
    return nc


def _ensure_ntff_hook():
    """Install the axon NTFF profile hook if the image lacks antenv.axon_hooks.

    Mirrors trn_agent_boot's ctypes hook against /opt/axon/libaxon_pjrt.so so
    run_bass_kernel_spmd(trace=True) can capture neuron-profile NTFF files.
    """
    import contextlib
    import ctypes
    import types

    try:
        from antenv.axon_hooks import get_axon_ntff_profile_hook  # noqa: F401
        return
    except ImportError:
        pass

    so_path = "/opt/axon/libaxon_pjrt.so"
    hook = None
    try:
        lib = ctypes.CDLL(so_path)
        if hasattr(lib, "axon_start_nrt_profile"):
            lib.axon_start_nrt_profile.argtypes = [
                ctypes.POINTER(ctypes.c_int64), ctypes.c_size_t]
            lib.axon_start_nrt_profile.restype = ctypes.c_int64
            lib.axon_stop_nrt_profile.argtypes = [ctypes.c_char_p]
            lib.axon_stop_nrt_profile.restype = ctypes.c_int64

            @contextlib.contextmanager
            def _hook(output_dir, device_ids):
                import jax
                jax.devices()
                if device_ids:
                    ids = (ctypes.c_int64 * len(device_ids))(*device_ids)
                    rc = lib.axon_start_nrt_profile(ids, len(device_ids))
                else:
                    rc = lib.axon_start_nrt_profile(None, 0)
                if rc != 0:
                    raise RuntimeError(f"axon_start_nrt_profile rc={rc}")
                try:
                    yield
                finally:
                    n = lib.axon_stop_nrt_profile(str(output_dir).encode())
                    print(f"profile: {n} file(s) written to {output_dir}")

            hook = _hook
    except OSError:
        pass

    import antenv

    mod = types.ModuleType("antenv.axon_hooks")
    mod._hook = hook
    mod.get_axon_ntff_profile_hook = lambda: mod._hook
    mod.set_axon_ntff_profile_hook = lambda h: setattr(mod, "_hook", h)
    sys.modules["antenv.axon_hooks"] = mod
    antenv.axon_hooks = mod


def _prep_core_inputs(s, z, y0, noiset_all, shared):
    """Slice + lay out the batch shard for core s (host-side np only)."""
    b0 = s * BC
    zs = z[b0:b0 + BC]                                   # [128, 325, 64]
    xtinit = np.zeros((65, 384, 128), np.float32)
    xtinit[0:64, :N, :] = zs.transpose(2, 1, 0)
    y0s = y0[b0:b0 + BC]                                 # [128, 325]
    xtinit[64, :N, :] = y0s.T
    y0T = np.zeros((NP, 128), np.float32)
    y0T[:N] = y0s.T
    y0t = np.zeros((128, 384), np.float32)
    for c in range(3):
        csz = CHP[c]
        y0t[:csz, 128 * c:128 * c + 128] = y0T[128 * c:128 * c + csz]
    noiset = np.zeros((T, NP, 128), np.float32)
    noiset[:, :N] = noiset_all[:, :, b0:b0 + BC]         # [T, 325, 128]
    nst = np.zeros((T, 128, 384), np.float32)
    for c in range(3):
        csz = CHP[c]
        nst[:, :csz, 128 * c:128 * c + 128] = noiset[:, 128 * c:128 * c + csz]
    bf = ml_dtypes.bfloat16
    m = {
        "xtinit": xtinit.reshape(65, 3 * 16384).astype(bf),
        "y0t": y0t,
        "noiset": nst,
    }
    m.update(shared)
    return m


def kernel(times, y0, z, noise, Wa, ba, Wb, bb, Wo, bo,
           GW0, Gb0, GW1, Gb1, GW2, Gb2, GWf, Gbf):
    times, y0, z, noise = (np.asarray(x, np.float32) for x in (times, y0, z, noise))
    Wa, ba, Wb, bb, Wo, bo = (np.asarray(x, np.float32) for x in (Wa, ba, Wb, bb, Wo, bo))
    GW0, Gb0, GW1, Gb1, GW2, Gb2, GWf, Gbf = (
        np.asarray(x, np.float32) for x in (GW0, Gb0, GW1, Gb1, GW2, Gb2, GWf, Gbf))

    dtv = float(times[1] - times[0])
    sqdt = float(np.sqrt(dtv))
    bf = ml_dtypes.bfloat16

    # fold wo*dt into Wa / ba / bo
    wa_f = Wa * (Wo[:, None, :] * dtv)                   # [T, 65, 64]
    ba_f = ba * Wo * dtv                                 # [T, 64]
    bo_dt = bo * dtv                                     # [T]

    wab = np.zeros((65, 128 * T), np.float32)
    bstack = np.zeros((128, 2 * T), np.float32)
    for k in range(T):
        wab[:, 128 * k:128 * k + 64] = wa_f[k]
        wab[:, 128 * k + 64:128 * k + 128] = Wb[k]
        bstack[0:64, 2 * k] = ba_f[k]
        bstack[64:128, 2 * k] = ba_f[k]
        bstack[0:64, 2 * k + 1] = bb[k]
        bstack[64:128, 2 * k + 1] = bb[k]

    gw0 = np.zeros((128, 192), np.float32)
    for c in range(3):
        csz = 128 if c < 2 else C2R
        gw0[:csz, 64 * c:64 * c + 64] = GW0[128 * c:128 * c + csz]
    gw12 = np.concatenate([GW1, GW2], axis=1)            # [64, 128]
    gwf = np.zeros((64, NP), np.float32)
    gwf[:, :N] = GWf
    gbm = np.zeros((128, 8), np.float32)
    gbm[0:64, 0], gbm[0:64, 1], gbm[0:64, 2] = Gb0, Gb1, Gb2
    for c in range(3):
        csz = 128 if c < 2 else C2R
        gbm[:csz, 3 + c] = Gbf[128 * c:128 * c + csz]
    ones2 = np.zeros((128, 32), np.float32)
    ones2[0:64, 0] = 1.0
    ones2[64:128, 1] = 1.0

    shared = {
        "wab": wab.astype(bf),
        "bstack": bstack,
        "gw0": gw0.astype(bf),
        "gw12": gw12.astype(bf),
        "gwf": gwf.astype(bf),
        "gb": gbm,
        "ones2": ones2.astype(bf),
    }

    noiset_all = noise.transpose(0, 2, 1)                # [T, 325, 1024]
    in_maps = [_prep_core_inputs(s, z, y0, noiset_all, shared)
               for s in range(NCORES)]

    nc = _build_program(bo_dt, sqdt)
    import os
    trace = bool(os.environ.get("BASS_KERNEL_PROFILE"))
    if trace:
        _ensure_ntff_hook()
    res = bass_utils.run_bass_kernel_spmd(nc, in_maps, core_ids=list(range(NCORES)),
                                          trace=trace)
    global LAST_RESULTS
    LAST_RESULTS = res

    out = np.empty((B, N), np.float32)
    for s in range(NCORES):
        yo = np.asarray(res.results[s]["yout"])          # [128, 384]
        yT = np.empty((N, BC), np.float32)
        for c in range(3):
            csz = 128 if c < 2 else C2R
            yT[128 * c:128 * c + csz] = yo[:csz, 128 * c:128 * c + 128]
        out[s * BC:(s + 1) * BC] = yT.T
    return out


# revision 16
# speedup vs baseline: 1.0655x; 1.0655x over previous
"""Trainium2 Bass kernel for a CSDE decoder (Euler-Maruyama scan with
step-aware GLU drift + diffusion MLP), data-parallel over batch on 8 cores.

Math per step k (reference):
    x   = concat(z, y[:, :, None], -1)                # [B,N,65]
    a   = x @ Wa_k + ba_k ;  b = x @ Wb_k + bb_k      # [B,N,64]
    f   = (a * sigmoid(b)) @ wo_k + bo_k              # [B,N]
    g   = tanh(relu(relu(relu(y@GW0+Gb0)@GW1+Gb1)@GW2+Gb2)@GWf+Gbf)
    y  <- y + f*dt + g*sqdt*eps_k

Device layout (per core, B_c=128 batch rows):
  - tokens = (node, batch) pairs, node-major columns: col = n*128 + b
  - xT resident in SBUF as [65, 328*128] bf16: rows 0-63 = z^T (static),
    row 64 = y broadcast row, re-written each step via a spread DMA.
  - wo*dt folded into Wa / ba / bo on the host.
  - per 8-node group: 4 matmuls (a/b for 2 half-groups stacked on
    partitions) -> sigmoid (ACT, full width) -> glu (DVE) -> block-diag
    ones matmul reduces glu over H to f for all 8 nodes.
  - g_func runs on y^T chunks ([node,batch] layout), all matmuls bf16.
  - y state kept fp32 in [128, 3*128] chunk layout; updates on DVE.
Host pre-transposes z / noise / y0 into these layouts (pure np prep).
"""

import sys

import numpy as np

sys.path.insert(0, "/opt/trn_rl_repo")

import ml_dtypes

import concourse.bass as bass
import concourse.bacc as bacc
import concourse.tile as tile
from concourse import mybir
from concourse import bass_utils
from contextlib import ExitStack

BF16 = mybir.dt.bfloat16
F32 = mybir.dt.float32

LAST_RESULTS = None  # BassKernelResults of the most recent kernel() call

B, N, H, HH, T = 1024, 325, 64, 64, 12
NCORES = 8
BC = B // NCORES          # 128 batch rows per core
NP = 328                  # nodes padded to 8*41
NG = NP // 8              # 41 groups of 8 nodes, exactly 16 per 128-node chunk
NPC = NP * 128            # valid xT columns
CHP = [128, 128, 72]      # packed chunk row counts (incl pad nodes)
C2R = N - 256             # 69 real rows in chunk 2


def _build_program(bo_dt: np.ndarray, sqdt: float):
    """Emit the full 12-step SPMD program for one core."""
    nc = bacc.Bacc("TRN2", target_bir_lowering=False, debug=False,
                   num_devices=NCORES)

    xtinit_d = nc.declare_dram_parameter("xtinit", [65, 3 * 16384], BF16, isOutput=False)
    y0t_d = nc.declare_dram_parameter("y0t", [128, 384], F32, isOutput=False)
    noiset_d = nc.declare_dram_parameter("noiset", [T, 128, 384], F32, isOutput=False)
    wab_d = nc.declare_dram_parameter("wab", [65, 128 * T], BF16, isOutput=False)
    bstack_d = nc.declare_dram_parameter("bstack", [128, 2 * T], F32, isOutput=False)
    gw0_d = nc.declare_dram_parameter("gw0", [128, 192], BF16, isOutput=False)
    gw12_d = nc.declare_dram_parameter("gw12", [64, 128], BF16, isOutput=False)
    gwf_d = nc.declare_dram_parameter("gwf", [64, NP], BF16, isOutput=False)
    gb_d = nc.declare_dram_parameter("gb", [128, 8], F32, isOutput=False)
    ones2_d = nc.declare_dram_parameter("ones2", [128, 32], BF16, isOutput=False)
    yout_d = nc.declare_dram_parameter("yout", [128, 384], F32, isOutput=True)

    Sig = mybir.ActivationFunctionType.Sigmoid
    Relu = mybir.ActivationFunctionType.Relu
    Tanh = mybir.ActivationFunctionType.Tanh
    ADD = mybir.AluOpType.add
    MULT = mybir.AluOpType.mult

    with ExitStack() as ctx:
        tc = ctx.enter_context(tile.TileContext(nc))
        pers = ctx.enter_context(tc.tile_pool(name="pers", bufs=1))

        xTs = [pers.tile([65, 16384], BF16, name=f"xT{c}") for c in range(3)]
        wab = pers.tile([65, 128 * T], BF16)
        bstack = pers.tile([128, 2 * T], F32)
        gw0 = pers.tile([128, 192], BF16)
        gw12 = pers.tile([64, 128], BF16)
        gwf = pers.tile([64, NP], BF16)
        gb = pers.tile([128, 8], F32)
        ones2 = pers.tile([128, 32], BF16)

        for c in range(3):
            nc.sync.dma_start(xTs[c][:, :], xtinit_d[:, 16384 * c:16384 * c + 16384])
        nc.sync.dma_start(wab[:, :], wab_d[:, :])
        nc.sync.dma_start(bstack[:, :], bstack_d[:, :])
        nc.sync.dma_start(gw0[:, :], gw0_d[:, :])
        nc.sync.dma_start(gw12[:, :], gw12_d[:, :])
        nc.sync.dma_start(gwf[:, :], gwf_d[:, :])
        nc.sync.dma_start(gb[:, :], gb_d[:, :])
        nc.sync.dma_start(ones2[:, :], ones2_d[:, :])

        ypool = ctx.enter_context(tc.tile_pool(name="ypool", bufs=2))
        ybfpool = ctx.enter_context(tc.tile_pool(name="ybfpool", bufs=2))
        epspool = ctx.enter_context(tc.tile_pool(name="epspool", bufs=3))
        fntpool = ctx.enter_context(tc.tile_pool(name="fntpool", bufs=2))
        gtpool = ctx.enter_context(tc.tile_pool(name="gtpool", bufs=6))
        hpool = ctx.enter_context(tc.tile_pool(name="hpool", bufs=3))
        sigpool = ctx.enter_context(tc.tile_pool(name="sigpool", bufs=3))
        glupool = ctx.enter_context(tc.tile_pool(name="glupool", bufs=3))
        fstpool = ctx.enter_context(tc.tile_pool(name="fstpool", bufs=3))
        upool = ctx.enter_context(tc.tile_pool(name="upool", bufs=4))

        papool = ctx.enter_context(tc.tile_pool(name="papool", bufs=2, space="PSUM"))  # 8 PSUM banks: pa2+pb2+pfa2+pfb1+pg1
        pbpool = ctx.enter_context(tc.tile_pool(name="pbpool", bufs=2, space="PSUM"))
        pfpool = ctx.enter_context(tc.tile_pool(name="pfpool", bufs=2, space="PSUM"))
        pgpool = ctx.enter_context(tc.tile_pool(name="pgpool", bufs=2, space="PSUM"))

        ycur = [ypool.tile([128, 128], F32, name=f"y_init{c}", tag=f"y{c}")
                for c in range(3)]
        ybf = [ybfpool.tile([128, 128], BF16, name=f"ybf_init{c}", tag=f"ybf{c}")
               for c in range(3)]
        for c in range(3):
            nc.sync.dma_start(ycur[c][:, :], y0t_d[:, 128 * c:128 * c + 128])
            nc.vector.tensor_copy(ybf[c][:, :], ycur[c][:, :])
        tc.strict_bb_all_engine_barrier()

        for k in range(T):
            eps = epspool.tile([128, 384], F32, name=f"eps{k}", tag="eps")
            nc.sync.dma_start(eps[:, :], noiset_d[k])
            fnts = [fntpool.tile([128, 128], F32, name=f"fnt{k}_{c}", tag=f"fnt{c}")
                    for c in range(3)]
            nc.vector.memset(fnts[2][64:128, :], 0.0)

            # ---- helpers emitted out-of-line for PE software pipelining ----
            def emit_gfunc():
                h0 = pgpool.tile([128, 128], F32, name=f"h0_{k}", tag="pg", bufs=1)
                nc.tensor.matmul(h0[0:64, :], gw0[0:128, 0:64], ybf[0][:, :],
                                 start=True, stop=False)
                nc.tensor.matmul(h0[0:64, :], gw0[0:128, 64:128], ybf[1][:, :],
                                 start=False, stop=False)
                nc.tensor.matmul(h0[0:64, :], gw0[0:C2R, 128:192],
                                 ybf[2][0:C2R, :], start=False, stop=True)
                h0s = hpool.tile([64, 128], BF16, name=f"h0s{k}", tag="hs")
                nc.scalar.activation(h0s[:, :], h0[0:64, :], Relu,
                                     bias=gb[0:64, 0:1])
                h1 = pgpool.tile([128, 128], F32, name=f"h1_{k}", tag="pg", bufs=1)
                nc.tensor.matmul(h1[0:64, :], gw12[:, 0:64], h0s[:, :])
                h1s = hpool.tile([64, 128], BF16, name=f"h1s{k}", tag="hs")
                nc.scalar.activation(h1s[:, :], h1[0:64, :], Relu,
                                     bias=gb[0:64, 1:2])
                h2 = pgpool.tile([128, 128], F32, name=f"h2_{k}", tag="pg", bufs=1)
                nc.tensor.matmul(h2[0:64, :], gw12[:, 64:128], h1s[:, :])
                h2s = hpool.tile([64, 128], BF16, name=f"h2s{k}", tag="hs")
                nc.scalar.activation(h2s[:, :], h2[0:64, :], Relu,
                                     bias=gb[0:64, 2:3])
                for c in range(3):
                    csz = 128 if c < 2 else C2R
                    gp = pgpool.tile([128, 128], F32, name=f"gp{k}_{c}",
                                     tag="pg", bufs=1)
                    nc.tensor.matmul(gp[0:csz, :], gwf[:, 128 * c:128 * c + csz],
                                     h2s[:, :])
                    gt = gtpool.tile([128, 128], F32, name=f"gt{k}_{c}", tag="gt")
                    if csz < 128:
                        nc.vector.memset(gt[64:128, :], 0.0)
                    nc.scalar.activation(gt[0:csz, :], gp[0:csz, :], Tanh,
                                         bias=gb[0:csz, 3 + c:4 + c])
                    gts.append(gt)

            def emit_ab(g):
                c = g // 16
                base = 1024 * (g % 16)
                blk = xTs[c][:, base:base + 1024].rearrange(
                    "p (n two b) -> p two n b", two=2, b=128)
                pa = papool.tile([128, 512], F32, name=f"pa{k}_{g}", tag="pa")
                pb = pbpool.tile([128, 512], F32, name=f"pb{k}_{g}", tag="pb")
                nc.tensor.matmul(pa[0:64, :], wa_k, blk[:, 0])
                nc.tensor.matmul(pa[64:128, :], wa_k, blk[:, 1])
                nc.tensor.matmul(pb[0:64, :], wb_k, blk[:, 0])
                nc.tensor.matmul(pb[64:128, :], wb_k, blk[:, 1])
                sig = sigpool.tile([128, 512], F32, name=f"sig{k}_{g}", tag="sig")
                nc.scalar.activation(sig[:, :], pb[:, :], Sig,
                                     bias=bstack[:, 2 * k + 1:2 * k + 2])
                glu = glupool.tile([128, 512], BF16, name=f"glu{k}_{g}", tag="glu")
                nc.vector.scalar_tensor_tensor(
                    glu[:, :], pa[:, :], bstack[:, 2 * k:2 * k + 1], sig[:, :],
                    op0=ADD, op1=MULT)
                return glu

            def emit_f(g, glu):
                c = g // 16
                pFa = pfpool.tile([128, 128], F32, name=f"pfa{k}_{g}", tag="pfa")
                pFb = pfpool.tile([128, 128], F32, name=f"pfb{k}_{g}", tag="pfb",
                                  bufs=1)
                for j in range(3):
                    nc.tensor.matmul(pFa[32 * j:32 * j + 32, :], ones2[:, :],
                                     glu[:, 128 * j:128 * j + 128])
                nc.tensor.matmul(pFb[0:32, :], ones2[:, :], glu[:, 384:512])
                fst = fstpool.tile([128, 128], F32, name=f"fst{k}_{g}", tag="fst")
                nc.scalar.copy(fst[0:66, :], pFa[0:66, :])
                fst2 = fstpool.tile([2, 128], F32, name=f"fst2_{k}_{g}", tag="fst2")
                nc.scalar.copy(fst2[0:2, :], pFb[0:2, :])
                r0 = 8 * (g % 16)
                for j in range(3):
                    nc.sync.dma_start(fnts[c][r0 + 2 * j:r0 + 2 * j + 2, :],
                                      fst[32 * j:32 * j + 2, :])
                nc.sync.dma_start(fnts[c][r0 + 6:r0 + 8, :], fst2[0:2, :])

            def emit_update(c):
                cs = slice(128 * c, 128 * c + 128)
                t1 = upool.tile([128, 128], F32, name=f"t1_{k}_{c}", tag="t1")
                nc.vector.tensor_mul(t1[:, :], gts[c][:, :], eps[:, cs])
                t2 = upool.tile([128, 128], F32, name=f"t2_{k}_{c}", tag="t2")
                nc.vector.scalar_tensor_tensor(
                    t2[:, :], t1[:, :], float(sqdt), ycur[c][:, :],
                    op0=MULT, op1=ADD)
                nc.vector.scalar_tensor_tensor(
                    ynext[c][:, :], fnts[c][:, :], float(bo_dt[k]), t2[:, :],
                    op0=ADD, op1=ADD)
                nc.vector.tensor_copy(ybfn[c][:, :], ynext[c][:, :])
                if k < T - 1:
                    nc.sync.dma_start(xTs[c][64:65, :], ybfn[c][:, :])

            # ---- PE-pipelined emission: ab(g) | f(g-1), updates inline ----
            wa_k = wab[:, 128 * k:128 * k + 64]
            wb_k = wab[:, 128 * k + 64:128 * k + 128]
            gts = []
            ynext = [ypool.tile([128, 128], F32, name=f"y{k + 1}_{c}", tag=f"y{c}")
                     for c in range(3)]
            ybfn = [ybfpool.tile([128, 128], BF16, name=f"ybf{k + 1}_{c}",
                                 tag=f"ybf{c}") for c in range(3)]
            glus = {}
            for g in range(NG):
                glus[g] = emit_ab(g)
                if g == 2:
                    emit_gfunc()
                if g >= 1:
                    emit_f(g - 1, glus.pop(g - 1))
                    if g - 1 == 15:
                        emit_update(0)
                    elif g - 1 == 31:
                        emit_update(1)
            emit_f(NG - 1, glus.pop(NG - 1))
            emit_update(2)
            ycur = ynext
            ybf = ybfn

        for c in range(3):
            nc.sync.dma_start(yout_d[:, 128 * c:128 * c + 128], ycur[c][:, :])

    nc.compile()
    return nc


def _ensure_ntff_hook():
    """Install the axon NTFF profile hook if the image lacks antenv.axon_hooks.

    Mirrors trn_agent_boot's ctypes hook against /opt/axon/libaxon_pjrt.so so
    run_bass_kernel_spmd(trace=True) can capture neuron-profile NTFF files.
    """
    import contextlib
    import ctypes
    import types

    try:
        from antenv.axon_hooks import get_axon_ntff_profile_hook  # noqa: F401
        return
    except ImportError:
        pass

    so_path = "/opt/axon/libaxon_pjrt.so"
    hook = None
    try:
        lib = ctypes.CDLL(so_path)
        if hasattr(lib, "axon_start_nrt_profile"):
            lib.axon_start_nrt_profile.argtypes = [
                ctypes.POINTER(ctypes.c_int64), ctypes.c_size_t]
            lib.axon_start_nrt_profile.restype = ctypes.c_int64
            lib.axon_stop_nrt_profile.argtypes = [ctypes.c_char_p]
            lib.axon_stop_nrt_profile.restype = ctypes.c_int64

            @contextlib.contextmanager
            def _hook(output_dir, device_ids):
                import jax
                jax.devices()
                if device_ids:
                    ids = (ctypes.c_int64 * len(device_ids))(*device_ids)
                    rc = lib.axon_start_nrt_profile(ids, len(device_ids))
                else:
                    rc = lib.axon_start_nrt_profile(None, 0)
                if rc != 0:
                    raise RuntimeError(f"axon_start_nrt_profile rc={rc}")
                try:
                    yield
                finally:
                    n = lib.axon_stop_nrt_profile(str(output_dir).encode())
                    print(f"profile: {n} file(s) written to {output_dir}")

            hook = _hook
    except OSError:
        pass

    import antenv

    mod = types.ModuleType("antenv.axon_hooks")
    mod._hook = hook
    mod.get_axon_ntff_profile_hook = lambda: mod._hook
    mod.set_axon_ntff_profile_hook = lambda h: setattr(mod, "_hook", h)
    sys.modules["antenv.axon_hooks"] = mod
    antenv.axon_hooks = mod


def _prep_core_inputs(s, z, y0, noiset_all, shared):
    """Slice + lay out the batch shard for core s (host-side np only)."""
    b0 = s * BC
    zs = z[b0:b0 + BC]                                   # [128, 325, 64]
    xtinit = np.zeros((65, 384, 128), np.float32)
    xtinit[0:64, :N, :] = zs.transpose(2, 1, 0)
    y0s = y0[b0:b0 + BC]                                 # [128, 325]
    xtinit[64, :N, :] = y0s.T
    y0T = np.zeros((NP, 128), np.float32)
    y0T[:N] = y0s.T
    y0t = np.zeros((128, 384), np.float32)
    for c in range(3):
        csz = CHP[c]
        y0t[:csz, 128 * c:128 * c + 128] = y0T[128 * c:128 * c + csz]
    noiset = np.zeros((T, NP, 128), np.float32)
    noiset[:, :N] = noiset_all[:, :, b0:b0 + BC]         # [T, 325, 128]
    nst = np.zeros((T, 128, 384), np.float32)
    for c in range(3):
        csz = CHP[c]
        nst[:, :csz, 128 * c:128 * c + 128] = noiset[:, 128 * c:128 * c + csz]
    bf = ml_dtypes.bfloat16
    m = {
        "xtinit": xtinit.reshape(65, 3 * 16384).astype(bf),
        "y0t": y0t,
        "noiset": nst,
    }
    m.update(shared)
    return m


def kernel(times, y0, z, noise, Wa, ba, Wb, bb, Wo, bo,
           GW0, Gb0, GW1, Gb1, GW2, Gb2, GWf, Gbf):
    times, y0, z, noise = (np.asarray(x, np.float32) for x in (times, y0, z, noise))
    Wa, ba, Wb, bb, Wo, bo = (np.asarray(x, np.float32) for x in (Wa, ba, Wb, bb, Wo, bo))
    GW0, Gb0, GW1, Gb1, GW2, Gb2, GWf, Gbf = (
        np.asarray(x, np.float32) for x in (GW0, Gb0, GW1, Gb1, GW2, Gb2, GWf, Gbf))

    dtv = float(times[1] - times[0])
    sqdt = float(np.sqrt(dtv))
    bf = ml_dtypes.bfloat16

    # fold wo*dt into Wa / ba / bo
    wa_f = Wa * (Wo[:, None, :] * dtv)                   # [T, 65, 64]
    ba_f = ba * Wo * dtv                                 # [T, 64]
    bo_dt = bo * dtv                                     # [T]

    wab = np.zeros((65, 128 * T), np.float32)
    bstack = np.zeros((128, 2 * T), np.float32)
    for k in range(T):
        wab[:, 128 * k:128 * k + 64] = wa_f[k]
        wab[:, 128 * k + 64:128 * k + 128] = Wb[k]
        bstack[0:64, 2 * k] = ba_f[k]
        bstack[64:128, 2 * k] = ba_f[k]
        bstack[0:64, 2 * k + 1] = bb[k]
        bstack[64:128, 2 * k + 1] = bb[k]

    gw0 = np.zeros((128, 192), np.float32)
    for c in range(3):
        csz = 128 if c < 2 else C2R
        gw0[:csz, 64 * c:64 * c + 64] = GW0[128 * c:128 * c + csz]
    gw12 = np.concatenate([GW1, GW2], axis=1)            # [64, 128]
    gwf = np.zeros((64, NP), np.float32)
    gwf[:, :N] = GWf
    gbm = np.zeros((128, 8), np.float32)
    gbm[0:64, 0], gbm[0:64, 1], gbm[0:64, 2] = Gb0, Gb1, Gb2
    for c in range(3):
        csz = 128 if c < 2 else C2R
        gbm[:csz, 3 + c] = Gbf[128 * c:128 * c + csz]
    ones2 = np.zeros((128, 32), np.float32)
    ones2[0:64, 0] = 1.0
    ones2[64:128, 1] = 1.0

    shared = {
        "wab": wab.astype(bf),
        "bstack": bstack,
        "gw0": gw0.astype(bf),
        "gw12": gw12.astype(bf),
        "gwf": gwf.astype(bf),
        "gb": gbm,
        "ones2": ones2.astype(bf),
    }

    noiset_all = noise.transpose(0, 2, 1)                # [T, 325, 1024]
    in_maps = [_prep_core_inputs(s, z, y0, noiset_all, shared)
               for s in range(NCORES)]

    nc = _build_program(bo_dt, sqdt)
    import os
    trace = bool(os.environ.get("BASS_KERNEL_PROFILE"))
    if trace:
        _ensure_ntff_hook()
    res = bass_utils.run_bass_kernel_spmd(nc, in_maps, core_ids=list(range(NCORES)),
                                          trace=trace)
    global LAST_RESULTS
    LAST_RESULTS = res

    out = np.empty((B, N), np.float32)
    for s in range(NCORES):
        yo = np.asarray(res.results[s]["yout"])          # [128, 384]
        yT = np.empty((N, BC), np.float32)
        for c in range(3):
            csz = 128 if c < 2 else C2R
            yT[128 * c:128 * c + csz] = yo[:csz, 128 * c:128 * c + 128]
        out[s * BC:(s + 1) * BC] = yT.T
    return out


# revision 18
# speedup vs baseline: 1.4598x; 1.3700x over previous
"""Trainium2 Bass kernel for a CSDE decoder (Euler-Maruyama scan with
step-aware GLU drift + diffusion MLP), data-parallel over batch on 8 cores.

Math per step k (reference):
    x   = concat(z, y[:, :, None], -1)                # [B,N,65]
    a   = x @ Wa_k + ba_k ;  b = x @ Wb_k + bb_k      # [B,N,64]
    f   = (a * sigmoid(b)) @ wo_k + bo_k              # [B,N]
    g   = tanh(relu(relu(relu(y@GW0+Gb0)@GW1+Gb1)@GW2+Gb2)@GWf+Gbf)
    y  <- y + f*dt + g*sqdt*eps_k

Device layout (per core, B_c=128 batch rows):
  - tokens = (node, batch) pairs, node-major columns: col = n*128 + b
  - xT resident in SBUF as [65, 328*128] bf16: rows 0-63 = z^T (static),
    row 64 = y broadcast row, re-written each step via a spread DMA.
  - wo*dt folded into Wa / ba / bo on the host.
  - per 8-node group: 4 matmuls (a/b for 2 half-groups stacked on
    partitions) -> sigmoid (ACT, full width) -> glu (DVE) -> block-diag
    ones matmul reduces glu over H to f for all 8 nodes.
  - g_func runs on y^T chunks ([node,batch] layout), all matmuls bf16.
  - y state kept fp32 in [128, 3*128] chunk layout; updates on DVE.
Host pre-transposes z / noise / y0 into these layouts (pure np prep).
"""

import sys

import numpy as np

sys.path.insert(0, "/opt/trn_rl_repo")

import ml_dtypes

import concourse.bass as bass
import concourse.bacc as bacc
import concourse.tile as tile
from concourse import mybir
from concourse import bass_utils
from contextlib import ExitStack

BF16 = mybir.dt.bfloat16
F32 = mybir.dt.float32

LAST_RESULTS = None  # BassKernelResults of the most recent kernel() call

B, N, H, HH, T = 1024, 325, 64, 64, 12
NCORES = 8
BC = B // NCORES          # 128 batch rows per core
NP = 328                  # nodes padded to 8*41
NG = NP // 8              # 41 groups of 8 nodes, exactly 16 per 128-node chunk
NPC = NP * 128            # valid xT columns
CHP = [128, 128, 72]      # packed chunk row counts (incl pad nodes)
C2R = N - 256             # 69 real rows in chunk 2


def _build_program(bo_dt: np.ndarray, sqdt: float):
    """Emit the full 12-step SPMD program for one core."""
    nc = bacc.Bacc("TRN2", target_bir_lowering=False, debug=False,
                   num_devices=NCORES)

    xtinit_d = nc.declare_dram_parameter("xtinit", [65, 3 * 16384], BF16, isOutput=False)
    y0t_d = nc.declare_dram_parameter("y0t", [128, 384], F32, isOutput=False)
    noiset_d = nc.declare_dram_parameter("noiset", [T, 128, 384], F32, isOutput=False)
    wab_d = nc.declare_dram_parameter("wab", [65, 128 * T], BF16, isOutput=False)
    bstack_d = nc.declare_dram_parameter("bstack", [128, 2 * T], F32, isOutput=False)
    gw0_d = nc.declare_dram_parameter("gw0", [128, 192], BF16, isOutput=False)
    gw12_d = nc.declare_dram_parameter("gw12", [64, 128], BF16, isOutput=False)
    gwf_d = nc.declare_dram_parameter("gwf", [64, NP], BF16, isOutput=False)
    gb_d = nc.declare_dram_parameter("gb", [128, 8], F32, isOutput=False)
    ones2_d = nc.declare_dram_parameter("ones2", [128, 32], BF16, isOutput=False)
    yout_d = nc.declare_dram_parameter("yout", [128, 384], F32, isOutput=True)

    Sig = mybir.ActivationFunctionType.Sigmoid
    Relu = mybir.ActivationFunctionType.Relu
    Tanh = mybir.ActivationFunctionType.Tanh
    ADD = mybir.AluOpType.add
    MULT = mybir.AluOpType.mult

    with ExitStack() as ctx:
        tc = ctx.enter_context(tile.TileContext(nc))
        pers = ctx.enter_context(tc.tile_pool(name="pers", bufs=1))

        xTs = [pers.tile([65, 16384], BF16, name=f"xT{c}") for c in range(3)]
        wab = pers.tile([65, 128 * T], BF16)
        bstack = pers.tile([128, 2 * T], F32)
        gw0 = pers.tile([128, 192], BF16)
        gw12 = pers.tile([64, 128], BF16)
        gwf = pers.tile([64, NP], BF16)
        gb = pers.tile([128, 8], F32)
        ones2 = pers.tile([128, 32], BF16)

        for c in range(3):
            nc.sync.dma_start(xTs[c][:, :], xtinit_d[:, 16384 * c:16384 * c + 16384])
        nc.sync.dma_start(wab[:, :], wab_d[:, :])
        nc.sync.dma_start(bstack[:, :], bstack_d[:, :])
        nc.sync.dma_start(gw0[:, :], gw0_d[:, :])
        nc.sync.dma_start(gw12[:, :], gw12_d[:, :])
        nc.sync.dma_start(gwf[:, :], gwf_d[:, :])
        nc.sync.dma_start(gb[:, :], gb_d[:, :])
        nc.sync.dma_start(ones2[:, :], ones2_d[:, :])

        ypool = ctx.enter_context(tc.tile_pool(name="ypool", bufs=2))
        ybfpool = ctx.enter_context(tc.tile_pool(name="ybfpool", bufs=2))
        epspool = ctx.enter_context(tc.tile_pool(name="epspool", bufs=3))
        fntpool = ctx.enter_context(tc.tile_pool(name="fntpool", bufs=2))
        gtpool = ctx.enter_context(tc.tile_pool(name="gtpool", bufs=6))
        hpool = ctx.enter_context(tc.tile_pool(name="hpool", bufs=3))
        sigpool = ctx.enter_context(tc.tile_pool(name="sigpool", bufs=3))
        glupool = ctx.enter_context(tc.tile_pool(name="glupool", bufs=3))
        fstpool = ctx.enter_context(tc.tile_pool(name="fstpool", bufs=3))
        upool = ctx.enter_context(tc.tile_pool(name="upool", bufs=4))

        papool = ctx.enter_context(tc.tile_pool(name="papool", bufs=2, space="PSUM"))  # 8 PSUM banks: pa2+pb2+pfa2+pfb1+pg1
        pbpool = ctx.enter_context(tc.tile_pool(name="pbpool", bufs=2, space="PSUM"))
        pfpool = ctx.enter_context(tc.tile_pool(name="pfpool", bufs=2, space="PSUM"))
        pgpool = ctx.enter_context(tc.tile_pool(name="pgpool", bufs=2, space="PSUM"))

        ycur = [ypool.tile([128, 128], F32, name=f"y_init{c}", tag=f"y{c}")
                for c in range(3)]
        ybf = [ybfpool.tile([128, 128], BF16, name=f"ybf_init{c}", tag=f"ybf{c}")
               for c in range(3)]
        for c in range(3):
            nc.sync.dma_start(ycur[c][:, :], y0t_d[:, 128 * c:128 * c + 128])
            nc.vector.tensor_copy(ybf[c][:, :], ycur[c][:, :])
        tc.strict_bb_all_engine_barrier()

        for k in range(T):
            eps = epspool.tile([128, 384], F32, name=f"eps{k}", tag="eps")
            nc.sync.dma_start(eps[:, :], noiset_d[k])
            fnts = [fntpool.tile([128, 128], F32, name=f"fnt{k}_{c}", tag=f"fnt{c}")
                    for c in range(3)]
            nc.vector.memset(fnts[2][64:128, :], 0.0)

            # ---- helpers emitted out-of-line for PE software pipelining ----
            def emit_gfunc():
                h0 = pgpool.tile([128, 128], F32, name=f"h0_{k}", tag="pg", bufs=1)
                nc.tensor.matmul(h0[0:64, :], gw0[0:128, 0:64], ybf[0][:, :],
                                 start=True, stop=False)
                nc.tensor.matmul(h0[0:64, :], gw0[0:128, 64:128], ybf[1][:, :],
                                 start=False, stop=False)
                nc.tensor.matmul(h0[0:64, :], gw0[0:C2R, 128:192],
                                 ybf[2][0:C2R, :], start=False, stop=True)
                h0s = hpool.tile([64, 128], BF16, name=f"h0s{k}", tag="hs")
                nc.scalar.activation(h0s[:, :], h0[0:64, :], Relu,
                                     bias=gb[0:64, 0:1])
                h1 = pgpool.tile([128, 128], F32, name=f"h1_{k}", tag="pg", bufs=1)
                nc.tensor.matmul(h1[0:64, :], gw12[:, 0:64], h0s[:, :])
                h1s = hpool.tile([64, 128], BF16, name=f"h1s{k}", tag="hs")
                nc.scalar.activation(h1s[:, :], h1[0:64, :], Relu,
                                     bias=gb[0:64, 1:2])
                h2 = pgpool.tile([128, 128], F32, name=f"h2_{k}", tag="pg", bufs=1)
                nc.tensor.matmul(h2[0:64, :], gw12[:, 64:128], h1s[:, :])
                h2s = hpool.tile([64, 128], BF16, name=f"h2s{k}", tag="hs")
                nc.scalar.activation(h2s[:, :], h2[0:64, :], Relu,
                                     bias=gb[0:64, 2:3])
                for c in range(3):
                    csz = 128 if c < 2 else C2R
                    gp = pgpool.tile([128, 128], F32, name=f"gp{k}_{c}",
                                     tag="pg", bufs=1)
                    nc.tensor.matmul(gp[0:csz, :], gwf[:, 128 * c:128 * c + csz],
                                     h2s[:, :])
                    gt = gtpool.tile([128, 128], F32, name=f"gt{k}_{c}", tag="gt")
                    if csz < 128:
                        nc.vector.memset(gt[64:128, :], 0.0)
                    nc.scalar.activation(gt[0:csz, :], gp[0:csz, :], Tanh,
                                         bias=gb[0:csz, 3 + c:4 + c])
                    gts.append(gt)

            def emit_ab(g):
                c = g // 16
                base = 1024 * (g % 16)
                blk = xTs[c][:, base:base + 1024].rearrange(
                    "p (n two b) -> p two n b", two=2, b=128)
                pa = papool.tile([128, 512], F32, name=f"pa{k}_{g}", tag="pa")
                pb = pbpool.tile([128, 512], F32, name=f"pb{k}_{g}", tag="pb")
                nc.tensor.matmul(pa[0:64, :], wa_k, blk[:, 0])
                nc.tensor.matmul(pa[64:128, :], wa_k, blk[:, 1])
                nc.tensor.matmul(pb[0:64, :], wb_k, blk[:, 0])
                nc.tensor.matmul(pb[64:128, :], wb_k, blk[:, 1])
                sig = sigpool.tile([128, 512], F32, name=f"sig{k}_{g}", tag="sig")
                nc.scalar.activation(sig[:, :], pb[:, :], Sig,
                                     bias=bstack[:, 2 * k + 1:2 * k + 2])
                glu = glupool.tile([128, 512], BF16, name=f"glu{k}_{g}", tag="glu")
                nc.vector.scalar_tensor_tensor(
                    glu[:, :], pa[:, :], bstack[:, 2 * k:2 * k + 1], sig[:, :],
                    op0=ADD, op1=MULT)
                return glu

            def emit_f(g, glu):
                c = g // 16
                pFa = pfpool.tile([128, 128], F32, name=f"pfa{k}_{g}", tag="pfa")
                pFb = pfpool.tile([128, 128], F32, name=f"pfb{k}_{g}", tag="pfb",
                                  bufs=1)
                for j in range(3):
                    nc.tensor.matmul(pFa[32 * j:32 * j + 32, :], ones2[:, :],
                                     glu[:, 128 * j:128 * j + 128])
                nc.tensor.matmul(pFb[0:32, :], ones2[:, :], glu[:, 384:512])
                fst = fstpool.tile([128, 128], F32, name=f"fst{k}_{g}", tag="fst")
                nc.scalar.copy(fst[0:66, :], pFa[0:66, :])
                fst2 = fstpool.tile([2, 128], F32, name=f"fst2_{k}_{g}", tag="fst2")
                nc.scalar.copy(fst2[0:2, :], pFb[0:2, :])
                r0 = 8 * (g % 16)
                nc.sync.dma_start(fnts[c][r0:r0 + 5:2, :], fst[0:66:32, :])
                nc.gpsimd.dma_start(fnts[c][r0 + 1:r0 + 6:2, :], fst[1:66:32, :])
                eng3 = nc.sync if g % 2 == 0 else nc.gpsimd
                eng3.dma_start(fnts[c][r0 + 6:r0 + 8, :], fst2[0:2, :])

            def emit_update(c):
                cs = slice(128 * c, 128 * c + 128)
                t1 = upool.tile([128, 128], F32, name=f"t1_{k}_{c}", tag="t1")
                nc.vector.tensor_mul(t1[:, :], gts[c][:, :], eps[:, cs])
                t2 = upool.tile([128, 128], F32, name=f"t2_{k}_{c}", tag="t2")
                nc.vector.scalar_tensor_tensor(
                    t2[:, :], t1[:, :], float(sqdt), ycur[c][:, :],
                    op0=MULT, op1=ADD)
                nc.vector.scalar_tensor_tensor(
                    ynext[c][:, :], fnts[c][:, :], float(bo_dt[k]), t2[:, :],
                    op0=ADD, op1=ADD)
                nc.vector.tensor_copy(ybfn[c][:, :], ynext[c][:, :])
                if k < T - 1:
                    nc.sync.dma_start(xTs[c][64:65, :], ybfn[c][:, :])

            # ---- PE-pipelined emission: ab(g) | f(g-1), updates inline ----
            wa_k = wab[:, 128 * k:128 * k + 64]
            wb_k = wab[:, 128 * k + 64:128 * k + 128]
            gts = []
            ynext = [ypool.tile([128, 128], F32, name=f"y{k + 1}_{c}", tag=f"y{c}")
                     for c in range(3)]
            ybfn = [ybfpool.tile([128, 128], BF16, name=f"ybf{k + 1}_{c}",
                                 tag=f"ybf{c}") for c in range(3)]
            glus = {}
            for g in range(NG):
                glus[g] = emit_ab(g)
                if g == 2:
                    emit_gfunc()
                if g >= 1:
                    emit_f(g - 1, glus.pop(g - 1))
                    if g - 1 == 15:
                        emit_update(0)
                    elif g - 1 == 31:
                        emit_update(1)
            emit_f(NG - 1, glus.pop(NG - 1))
            emit_update(2)
            ycur = ynext
            ybf = ybfn

        for c in range(3):
            nc.sync.dma_start(yout_d[:, 128 * c:128 * c + 128], ycur[c][:, :])

    nc.compile()
    return nc


def _ensure_ntff_hook():
    """Install the axon NTFF profile hook if the image lacks antenv.axon_hooks.

    Mirrors trn_agent_boot's ctypes hook against /opt/axon/libaxon_pjrt.so so
    run_bass_kernel_spmd(trace=True) can capture neuron-profile NTFF files.
    """
    import contextlib
    import ctypes
    import types

    try:
        from antenv.axon_hooks import get_axon_ntff_profile_hook  # noqa: F401
        return
    except ImportError:
        pass

    so_path = "/opt/axon/libaxon_pjrt.so"
    hook = None
    try:
        lib = ctypes.CDLL(so_path)
        if hasattr(lib, "axon_start_nrt_profile"):
            lib.axon_start_nrt_profile.argtypes = [
                ctypes.POINTER(ctypes.c_int64), ctypes.c_size_t]
            lib.axon_start_nrt_profile.restype = ctypes.c_int64
            lib.axon_stop_nrt_profile.argtypes = [ctypes.c_char_p]
            lib.axon_stop_nrt_profile.restype = ctypes.c_int64

            @contextlib.contextmanager
            def _hook(output_dir, device_ids):
                import jax
                jax.devices()
                if device_ids:
                    ids = (ctypes.c_int64 * len(device_ids))(*device_ids)
                    rc = lib.axon_start_nrt_profile(ids, len(device_ids))
                else:
                    rc = lib.axon_start_nrt_profile(None, 0)
                if rc != 0:
                    raise RuntimeError(f"axon_start_nrt_profile rc={rc}")
                try:
                    yield
                finally:
                    n = lib.axon_stop_nrt_profile(str(output_dir).encode())
                    print(f"profile: {n} file(s) written to {output_dir}")

            hook = _hook
    except OSError:
        pass

    import antenv

    mod = types.ModuleType("antenv.axon_hooks")
    mod._hook = hook
    mod.get_axon_ntff_profile_hook = lambda: mod._hook
    mod.set_axon_ntff_profile_hook = lambda h: setattr(mod, "_hook", h)
    sys.modules["antenv.axon_hooks"] = mod
    antenv.axon_hooks = mod


def _prep_core_inputs(s, z, y0, noiset_all, shared):
    """Slice + lay out the batch shard for core s (host-side np only)."""
    b0 = s * BC
    zs = z[b0:b0 + BC]                                   # [128, 325, 64]
    xtinit = np.zeros((65, 384, 128), np.float32)
    xtinit[0:64, :N, :] = zs.transpose(2, 1, 0)
    y0s = y0[b0:b0 + BC]                                 # [128, 325]
    xtinit[64, :N, :] = y0s.T
    y0T = np.zeros((NP, 128), np.float32)
    y0T[:N] = y0s.T
    y0t = np.zeros((128, 384), np.float32)
    for c in range(3):
        csz = CHP[c]
        y0t[:csz, 128 * c:128 * c + 128] = y0T[128 * c:128 * c + csz]
    noiset = np.zeros((T, NP, 128), np.float32)
    noiset[:, :N] = noiset_all[:, :, b0:b0 + BC]         # [T, 325, 128]
    nst = np.zeros((T, 128, 384), np.float32)
    for c in range(3):
        csz = CHP[c]
        nst[:, :csz, 128 * c:128 * c + 128] = noiset[:, 128 * c:128 * c + csz]
    bf = ml_dtypes.bfloat16
    m = {
        "xtinit": xtinit.reshape(65, 3 * 16384).astype(bf),
        "y0t": y0t,
        "noiset": nst,
    }
    m.update(shared)
    return m


def kernel(times, y0, z, noise, Wa, ba, Wb, bb, Wo, bo,
           GW0, Gb0, GW1, Gb1, GW2, Gb2, GWf, Gbf):
    times, y0, z, noise = (np.asarray(x, np.float32) for x in (times, y0, z, noise))
    Wa, ba, Wb, bb, Wo, bo = (np.asarray(x, np.float32) for x in (Wa, ba, Wb, bb, Wo, bo))
    GW0, Gb0, GW1, Gb1, GW2, Gb2, GWf, Gbf = (
        np.asarray(x, np.float32) for x in (GW0, Gb0, GW1, Gb1, GW2, Gb2, GWf, Gbf))

    dtv = float(times[1] - times[0])
    sqdt = float(np.sqrt(dtv))
    bf = ml_dtypes.bfloat16

    # fold wo*dt into Wa / ba / bo
    wa_f = Wa * (Wo[:, None, :] * dtv)                   # [T, 65, 64]
    ba_f = ba * Wo * dtv                                 # [T, 64]
    bo_dt = bo * dtv                                     # [T]

    wab = np.zeros((65, 128 * T), np.float32)
    bstack = np.zeros((128, 2 * T), np.float32)
    for k in range(T):
        wab[:, 128 * k:128 * k + 64] = wa_f[k]
        wab[:, 128 * k + 64:128 * k + 128] = Wb[k]
        bstack[0:64, 2 * k] = ba_f[k]
        bstack[64:128, 2 * k] = ba_f[k]
        bstack[0:64, 2 * k + 1] = bb[k]
        bstack[64:128, 2 * k + 1] = bb[k]

    gw0 = np.zeros((128, 192), np.float32)
    for c in range(3):
        csz = 128 if c < 2 else C2R
        gw0[:csz, 64 * c:64 * c + 64] = GW0[128 * c:128 * c + csz]
    gw12 = np.concatenate([GW1, GW2], axis=1)            # [64, 128]
    gwf = np.zeros((64, NP), np.float32)
    gwf[:, :N] = GWf
    gbm = np.zeros((128, 8), np.float32)
    gbm[0:64, 0], gbm[0:64, 1], gbm[0:64, 2] = Gb0, Gb1, Gb2
    for c in range(3):
        csz = 128 if c < 2 else C2R
        gbm[:csz, 3 + c] = Gbf[128 * c:128 * c + csz]
    ones2 = np.zeros((128, 32), np.float32)
    ones2[0:64, 0] = 1.0
    ones2[64:128, 1] = 1.0

    shared = {
        "wab": wab.astype(bf),
        "bstack": bstack,
        "gw0": gw0.astype(bf),
        "gw12": gw12.astype(bf),
        "gwf": gwf.astype(bf),
        "gb": gbm,
        "ones2": ones2.astype(bf),
    }

    noiset_all = noise.transpose(0, 2, 1)                # [T, 325, 1024]
    in_maps = [_prep_core_inputs(s, z, y0, noiset_all, shared)
               for s in range(NCORES)]

    nc = _build_program(bo_dt, sqdt)
    import os
    trace = bool(os.environ.get("BASS_KERNEL_PROFILE"))
    if trace:
        _ensure_ntff_hook()
    res = bass_utils.run_bass_kernel_spmd(nc, in_maps, core_ids=list(range(NCORES)),
                                          trace=trace)
    global LAST_RESULTS
    LAST_RESULTS = res

    out = np.empty((B, N), np.float32)
    for s in range(NCORES):
        yo = np.asarray(res.results[s]["yout"])          # [128, 384]
        yT = np.empty((N, BC), np.float32)
        for c in range(3):
            csz = 128 if c < 2 else C2R
            yT[128 * c:128 * c + csz] = yo[:csz, 128 * c:128 * c + 128]
        out[s * BC:(s + 1) * BC] = yT.T
    return out
